# revision 1
# baseline (speedup 1.0000x reference)
"""GQA causal attention block (x @ Wq/Wk/Wv -> causal GQA attention -> @ Wo)
for Trainium2, SPMD over 8 NeuronCores.

Sharding: 4 batches x 2 query-shards. Core c handles batch c//2 and the
interleaved set of 128-row query tiles {s, s+2, ...} (s = c%2), which
balances the causal-attention triangle between the two shards of a batch.

vs. the v1 kernel:
- k/v are projected only for the core's own rows; the two cores of a
  batch swap halves through two 8-core AllGathers (one per 512-row
  chunk, launched as soon as that chunk's k/v is ready, running on
  TOPSP/SDMA fully overlapped with q-projection).
- x is transposed once (own rows only) and reused for q-projection.
- Wq is streamed once per head-group, Wo once; o-projection runs in a
  single pass using all 8 PSUM banks.
- softmax reciprocals use the fast approximate DVE op (~5x cheaper).

The attention inner loop keeps the dense 3-matmul form (scores,
ones-rowsum, p@V) of v1: the PE clock throttles down when its duty
cycle drops, so "saving" the rowsum matmul makes everything slower.

Key-slot layout keeps the SPMD program shard-independent: slots 0..7
hold the core's own key tiles (local order), slots 8..15 the
partner's. Query tile j attends over slots {0..j} u {8..8+j}; the
host-provided masks make it causal: masks[0] (slot j, the own-side
diagonal) is triangular for both shards, masks[1] (slot 8+j) is -inf
for shard 0 (future keys) and 0 for shard 1 (past keys). The partner
block's position in the AllGather output is the only rank-dependent
address, supplied per-core as a uint32 element offset ("poff") and
used as a runtime DMA offset register.
"""

import sys

for _p in ("/opt/trn_rl_repo", "/root/.axon_site/_ro/trn_rl_repo"):
    if _p not in sys.path:
        sys.path.append(_p)

import numpy as np
import ml_dtypes

import concourse.bacc as bacc
import concourse.bass as bass
import concourse.tile as tile
import concourse.mybir as mybir
from concourse.bass_utils import run_bass_kernel_spmd

F32 = mybir.dt.float32
BF16 = mybir.dt.bfloat16
FP16 = mybir.dt.float16
U32 = mybir.dt.uint32
AF = mybir.ActivationFunctionType
NEG = -1.0e6  # additive mask for disallowed keys (pre-softmax-scale)


class Cfg:
    def __init__(self, T, E, H, KV, n_batch, n_shard):
        self.T, self.E, self.H, self.KV = T, E, H, KV
        self.D = 128
        self.G4 = H // 4             # 4-head kv groups
        self.NE = E // 128           # contraction chunks for projections
        self.n_batch = n_batch
        self.n_shard = n_shard
        self.n_cores = n_batch * n_shard
        self.RQ = T // n_shard       # query rows per core
        self.NJ = self.RQ // 128     # local 128-row query tiles
        self.NLT = self.RQ // 512    # local 512-row chunks
        self.NT = T // 128           # global 128-row tiles
        self.HKV = KV * self.D       # k/v projection width
        self.scale = 1.0 / float(np.sqrt(self.D))
        # per-chunk exchange block: kT (KV heads) + v (4 local tiles)
        self.CCB = (self.KV + 4) * 128  # rows per cc_in buffer


FULL = Cfg(T=2048, E=2048, H=16, KV=4, n_batch=4, n_shard=2)


def build(cfg):
    c = cfg
    nc = bacc.Bacc("TRN2", target_bir_lowering=False, debug=False,
                   num_devices=c.n_cores)

    xq_d = nc.dram_tensor("xq", [c.RQ, c.E], BF16, kind="ExternalInput").ap()
    wq_d = nc.dram_tensor("Wq", [c.E, c.H * c.D], BF16, kind="ExternalInput").ap()
    wk_d = nc.dram_tensor("Wk", [c.E, c.HKV], BF16, kind="ExternalInput").ap()
    wv_d = nc.dram_tensor("Wv", [c.E, c.HKV], BF16, kind="ExternalInput").ap()
    wo_d = nc.dram_tensor("Wo", [c.H * c.D, c.E], BF16, kind="ExternalInput").ap()
    mask_d = nc.dram_tensor("masks", [2, 128, 512], F32,
                            kind="ExternalInput").ap()
    idb_d = nc.dram_tensor("identb", [128, 128], BF16, kind="ExternalInput").ap()
    onesh_d = nc.dram_tensor("onesh", [128, 128], FP16, kind="ExternalInput").ap()
    poff_d = nc.dram_tensor("poff", [1, 1], U32, kind="ExternalInput").ap()
    o_d = nc.dram_tensor("o", [c.RQ, c.E], F32, kind="ExternalOutput").ap()

    from contextlib import ExitStack
    with tile.TileContext(nc) as tc:
        with ExitStack() as _st:
            def pool(name, bufs, space="SBUF"):
                return _st.enter_context(
                    tc.tile_pool(name=name, bufs=bufs, space=space))
            constp = pool("const", 1)
            xqtp = pool("xqt", c.NE)
            ktp = pool("kts", c.KV * 4)
            vp = pool("vsb", c.NT)
            qtp = pool("qt", 16)
            ytp = pool("yt", c.G4 * c.NJ)
            wqp = pool("wq", c.NE)
            wkvp = pool("wkv", 6)
            wop = pool("wo", 8)
            smp = pool("sm", 6)
            accp = pool("accp", 4)
            bsbp = pool("bsb", 4)
            xnp = pool("xn", 8)
            osbp = pool("osb", 6)
            pq = pool("pq", 2, space="PSUM")
            pa = pool("pa", 2, space="PSUM")
            py = pool("py", 2, space="PSUM")
            dramp = pool("dram", 1, space="DRAM")

            # --- constants (identb first: the warmup needs it; the rest go
            # on the scalar queue so they don't delay the first x tiles) ---
            identb = constp.tile([128, 128], BF16, tag="identb")
            nc.sync.dma_start(identb[:], idb_d[:])
            masks = []
            for i in range(2):
                m = constp.tile([128, 512], F32, tag=f"mask{i}", name=f"mask{i}")
                nc.sync.dma_start(m[:], mask_d[i])
                masks.append(m)
            onesh = constp.tile([128, 128], FP16, tag="onesh")
            nc.sync.dma_start(onesh[:], onesh_d[:])
            poffs = constp.tile([1, 1], U32, tag="poffs")
            nc.sync.dma_start(poffs[:], poff_d[:])

            cc_in = [dramp.tile([c.CCB, 512], BF16, name=f"cc_in{lt}",
                                tag=f"cc_in{lt}") for lt in range(c.NLT)]
            cc_out = [dramp.tile([2 * c.CCB, 512], BF16,
                                 name=f"cc_out{lt}",
                                 tag=f"cc_out{lt}") for lt in range(c.NLT)]

            # warm the PE clock-gate during the initial DMA ramp
            pwu = pa.tile([128, 512], BF16, tag="pa", name="pwu")
            for wu in range(24):
                nc.tensor.transpose(pwu[:, (wu % 4) * 128:(wu % 4 + 1) * 128],
                                    identb[:], identb[:])

            # persistent activations
            xqT = [xqtp.tile([128, c.RQ], BF16, tag="xqT", name=f"xqT{e}")
                   for e in range(c.NE)]
            kts = [[ktp.tile([128, 512], BF16, tag="kts", name=f"kts{h}_{q}")
                    for q in range(4)] for h in range(c.KV)]
            v_sb = [vp.tile([128, c.HKV], BF16, tag="v", name=f"v{i}")
                    for i in range(c.NT)]

            # partner block offset (elements) comes from host data
            poff_r = nc.gpsimd.alloc_register("poff_r")
            nc.gpsimd.reg_load(poff_r, poffs[0:1, 0:1])
            poff_v = nc.gpsimd.snap(poff_r, donate=True, min_val=0,
                                    max_val=c.CCB * 512)

            def cc_src(lt, block):
                off = poff_v + block * 128 * 512
                return bass.AP(cc_out[lt].tensor, off, [[512, 128], [1, 512]])

            # ---------------- Phase A: transposes + own-half k/v ------------
            def phase_a(lt):
                # transpose own 512 rows into xqT[e][:, lt*512:(lt+1)*512]
                for qa in range(c.NE // 4):
                    xns = []
                    for i in range(4):
                        xn = xnp.tile([128, 512], BF16, tag="xn",
                                      name=f"xn{i}")
                        nc.sync.dma_start(
                            xn[:], xq_d[lt * 512 + i * 128:
                                        lt * 512 + (i + 1) * 128,
                                        qa * 512:(qa + 1) * 512])
                        xns.append(xn)
                    for eh in range(4):
                        e = qa * 4 + eh
                        ptr = pa.tile([128, 512], BF16, tag="pa", name="ptr")
                        for i in range(4):
                            nc.tensor.transpose(
                                ptr[:, i * 128:(i + 1) * 128],
                                xns[i][:, eh * 128:(eh + 1) * 128], identb[:])
                        nc.vector.tensor_copy(
                            xqT[e][:, lt * 512:(lt + 1) * 512], ptr[:])

                # kT for own rows -> slots 4*lt..4*lt+3 (= quad lt)
                psk = ([pq.tile([128, 512], F32, tag="pq", name=f"psk{h}")
                        for h in range(2)] +
                       [pa.tile([128, 512], F32, tag="pa", name=f"psk{h + 2}")
                        for h in range(2)])
                for e in range(c.NE):
                    wk_t = wkvp.tile([128, c.HKV], BF16, tag="wkv", name="wk_t")
                    nc.gpsimd.dma_start(wk_t[:], wk_d[e * 128:(e + 1) * 128, :])
                    for h in range(c.KV):
                        nc.tensor.matmul(psk[h][:],
                                         wk_t[:, h * 128:(h + 1) * 128],
                                         xqT[e][:, lt * 512:(lt + 1) * 512],
                                         start=(e == 0), stop=(e == c.NE - 1))
                for h in range(c.KV):
                    nc.vector.tensor_copy(kts[h][lt][:], psk[h][:])
                    nc.gpsimd.dma_start(
                        cc_in[lt][h * 128:(h + 1) * 128, :], kts[h][lt][:])

                # v for own rows -> slots 4*lt..4*lt+3
                psv = ([pq.tile([128, c.HKV], F32, tag="pq", name=f"psv{i}")
                        for i in range(2)] +
                       [pa.tile([128, c.HKV], F32, tag="pa", name=f"psv{i + 2}")
                        for i in range(2)])
                for e in range(c.NE):
                    wv_t = wkvp.tile([128, c.HKV], BF16, tag="wkv", name="wv_t")
                    nc.gpsimd.dma_start(wv_t[:], wv_d[e * 128:(e + 1) * 128, :])
                    for i in range(4):
                        nc.tensor.matmul(psv[i][:],
                                         xqT[e][:, lt * 512 + i * 128:
                                                lt * 512 + (i + 1) * 128],
                                         wv_t[:],
                                         start=(e == 0), stop=(e == c.NE - 1))
                for i in range(4):
                    sl = lt * 4 + i
                    nc.vector.tensor_copy(v_sb[sl][:], psv[i][:])
                    nc.gpsimd.dma_start(
                        cc_in[lt][(c.KV + i) * 128:(c.KV + i + 1) * 128, :],
                        v_sb[sl][:])

            def launch_ag(lt):
                nc.gpsimd.collective_compute(
                    "AllGather",
                    mybir.AluOpType.bypass,
                    replica_groups=[[2 * p, 2 * p + 1]
                                    for p in range(c.n_cores // 2)],
                    ins=[cc_in[lt].opt()],
                    outs=[cc_out[lt].opt()],
                )

            def unpack(lt):
                for h in range(c.KV):
                    nc.gpsimd.dma_start(kts[h][2 + lt][:], cc_src(lt, h))
                for i in range(4):
                    nc.gpsimd.dma_start(v_sb[8 + lt * 4 + i][:],
                                        cc_src(lt, c.KV + i))

            phase_a(0)
            launch_ag(0)
            phase_a(1)
            launch_ag(1)
            unpack(0)
            unpack(1)

            # ---------------- q-projection for one group --------------------
            # generator: yields once per PE matmul so attention can consume
            # it as PE filler between scalar-bound softmax pairs
            def q_proj_gen(g, out):
                wqt = []
                for e in range(c.NE):
                    w = wqp.tile([128, 512], BF16, tag="wq", name=f"wq{e}")
                    nc.sync.dma_start(
                        w[:], wq_d[e * 128:(e + 1) * 128,
                                   g * 512:(g + 1) * 512])
                    wqt.append(w)
                for blk in range(2):
                    qs = [qtp.tile([128, 512], BF16, tag="qT",
                                   name=f"qT{g}_{blk}_{jj}")
                          for jj in range(4)]
                    for hp in range(2):
                        psq = [pq.tile([128, 512], F32, tag="pq",
                                       name=f"psq{i}") for i in range(2)]
                        for e in range(c.NE):
                            for hi in range(2):
                                hh = hp * 2 + hi
                                nc.tensor.matmul(
                                    psq[hi][:],
                                    wqt[e][:, hh * 128:(hh + 1) * 128],
                                    xqT[e][:, blk * 512:(blk + 1) * 512],
                                    start=(e == 0), stop=(e == c.NE - 1))
                                yield
                        for jj in range(4):
                            for hi in range(2):
                                hh = hp * 2 + hi
                                nc.vector.tensor_copy(
                                    qs[jj][:, hh * 128:(hh + 1) * 128],
                                    psq[hi][:, jj * 128:(jj + 1) * 128])
                    out.extend(qs)

            def drain(gen):
                if gen is not None:
                    for _ in gen:
                        pass

            # ---------------- attention for one group -----------------------
            # key tiles are processed in pairs: two score matmuls fill the
            # halves of one 2-bank PSUM tile, ONE exp covers both (the 185ns
            # activation access latency amortizes), softmax denominators
            # accumulate on the vector engine (fp16) with a single
            # ones-matmul per query tile. The filler generator keeps the PE
            # dense while the scalar engine paces the softmax.
            def slot(kk, j):
                return kk if kk <= j else 8 + (kk - j - 1)

            def attention(g, qT, filler):
                for j in range(c.NJ):
                    nk = 2 * (j + 1)
                    psy = py.tile([128, 512], F32, tag="py", name="psy")
                    acc = accp.tile([128, 512], FP16, tag="acc", name="acc")
                    for p in range(j + 1):
                        k0, k1 = 2 * p, 2 * p + 1
                        sl0, sl1 = slot(k0, j), slot(k1, j)
                        sct2 = pa.tile([128, 1024], F32, tag="pa",
                                       name="sct2")
                        for half, sl in ((0, sl0), (1, sl1)):
                            nc.tensor.matmul(
                                sct2[:, half * 512:(half + 1) * 512],
                                kts[g][sl // 4][:, (sl % 4) * 128:
                                                (sl % 4 + 1) * 128],
                                qT[j][:],
                                start=True, stop=True)
                        for half, kk in ((0, k0), (1, k1)):
                            if kk == j:
                                nc.vector.tensor_add(
                                    sct2[:, half * 512:(half + 1) * 512],
                                    sct2[:, half * 512:(half + 1) * 512],
                                    masks[0][:])
                            elif kk == nk - 1:
                                nc.vector.tensor_add(
                                    sct2[:, half * 512:(half + 1) * 512],
                                    sct2[:, half * 512:(half + 1) * 512],
                                    masks[1][:])
                        pbt2 = smp.tile([128, 1024], BF16, tag="pbt",
                                        name="pbt2")
                        nc.scalar.activation(pbt2[:], sct2[:], AF.Exp,
                                             scale=c.scale)
                        if p == 0:
                            nc.vector.tensor_copy(acc[:], pbt2[:, 0:512])
                        else:
                            nc.vector.tensor_add(acc[:], acc[:],
                                                 pbt2[:, 0:512])
                        nc.vector.tensor_add(acc[:], acc[:],
                                             pbt2[:, 512:1024])
                        nc.tensor.matmul(
                            psy[:],
                            v_sb[sl0][:, g * 128:(g + 1) * 128],
                            pbt2[:, 0:512],
                            start=(p == 0), stop=False)
                        nc.tensor.matmul(
                            psy[:],
                            v_sb[sl1][:, g * 128:(g + 1) * 128],
                            pbt2[:, 512:1024],
                            start=False, stop=(p == j))
                        if filler is not None:
                            next(filler, None)
                    psums = pa.tile([128, 512], F32, tag="pa", name="psums")
                    nc.tensor.matmul(psums[:], onesh[:], acc[:],
                                     start=True, stop=True)
                    bsb = bsbp.tile([128, 512], F32, tag="bsb", name="bsb")
                    nc.vector.reciprocal_approx_fast(bsb[:], psums[:])
                    yt = ytp.tile([128, 512], BF16, tag="yT",
                                  name=f"yT{g}_{j}")
                    nc.vector.tensor_mul(yt[:], psy[:], bsb[:])
                    yT[g][j] = yt

            # two-group software pipeline: groups 0/1 are projected up
            # front (covering the AllGather window); groups 2/3 stream in
            # as PE filler inside the attention of groups 0/1
            yT = [[None] * c.NJ for _ in range(c.G4)]
            qTs = [[] for _ in range(c.G4)]
            drain(q_proj_gen(0, qTs[0]))
            drain(q_proj_gen(1, qTs[1]))
            fillers = [q_proj_gen(2, qTs[2]), q_proj_gen(3, qTs[3]),
                       None, None]
            for g in range(c.G4):
                attention(g, qTs[g], fillers[g])
                drain(fillers[g])

            # ---------------- Phase C: o-projection, single pass ------------
            for et in range(c.E // 512):
                pso2 = [pa.tile([128, 1024], F32, tag="pa",
                                name=f"pso2_{i}") for i in range(2)]
                pso = ([pq.tile([128, 512], F32, tag="pq", name=f"pso{i}")
                        for i in range(2)] +
                       [pso2[i][:, half * 512:(half + 1) * 512]
                        for i in range(2) for half in range(2)] +
                       [py.tile([128, 512], F32, tag="py", name=f"pso{i + 6}")
                        for i in range(2)])
                for h in range(c.H):
                    g, hh = divmod(h, 4)
                    wo_t = wop.tile([128, 512], BF16, tag="wo", name="wo_t")
                    nc.gpsimd.dma_start(
                        wo_t[:], wo_d[h * 128:(h + 1) * 128,
                                      et * 512:(et + 1) * 512])
                    for tsub in range(c.NJ):
                        nc.tensor.matmul(
                            pso[tsub][:],
                            yT[g][tsub][:, hh * 128:(hh + 1) * 128],
                            wo_t[:],
                            start=(h == 0), stop=(h == c.H - 1))
                for tsub in range(c.NJ):
                    osb = osbp.tile([128, 512], F32, tag="osb", name="osb")
                    nc.scalar.copy(osb[:], pso[tsub][:])
                    nc.sync.dma_start(o_d[tsub * 128:(tsub + 1) * 128,
                                          et * 512:(et + 1) * 512],
                                      osb[:])

    nc.compile()
    return nc


def make_masks(cfg, s):
    """Additive causal masks in scoresT ([key, query]) orientation, tiled
    4x along the free axis for the 4-head packing.

    masks[0] is added on the own-side diagonal slot (slot j): triangular
    keep k <= q for both shards. masks[1] is added on the partner-side
    final slot (slot 8+j): for shard 0 the partner tile holds future keys
    (drop all), for shard 1 past keys (keep all).
    """
    r = np.arange(128)
    triT = np.where(r[:, None] <= r[None, :], 0.0, NEG).astype(np.float32)
    out = np.zeros((2, 128, 128), np.float32)
    out[0] = triT
    if s == 0:
        out[1] = NEG
    return np.tile(out, (1, 1, 4))


def make_inputs(cfg, x, Wq, Wk, Wv, Wo):
    """Per-core input maps from full tensors (activations/weights in bf16)."""
    bf = ml_dtypes.bfloat16
    ident_b = np.eye(128, dtype=bf)
    ones_h = np.ones((128, 128), np.float16)
    Wqb, Wkb, Wvb, Wob = (np.asarray(w).astype(bf) for w in (Wq, Wk, Wv, Wo))
    in_maps = []
    for cc in range(cfg.n_cores):
        b, s = divmod(cc, cfg.n_shard)
        xb = np.asarray(x[b]).astype(bf)
        xq = np.ascontiguousarray(
            xb.reshape(cfg.T // 128, 128, cfg.E)[s::cfg.n_shard]
            .reshape(cfg.RQ, cfg.E))
        poff = np.array([[((cc & 1) ^ 1) * cfg.CCB * 512]], np.uint32)
        in_maps.append({
            "xq": xq, "Wq": Wqb, "Wk": Wkb, "Wv": Wvb, "Wo": Wob,
            "masks": make_masks(cfg, s),
            "identb": ident_b,
            "onesh": ones_h,
            "poff": poff,
        })
    return in_maps


def scatter_out(cfg, results):
    B = cfg.n_batch
    out = np.empty((B, cfg.T, cfg.E), np.float32)
    for cc in range(cfg.n_cores):
        b, s = divmod(cc, cfg.n_shard)
        out[b].reshape(cfg.T // 128, 128, cfg.E)[s::cfg.n_shard] = \
            results[cc]["o"].reshape(cfg.RQ // 128, 128, cfg.E)
    return out


_NC_CACHE = {}


def get_nc(cfg):
    key = (cfg.T, cfg.E, cfg.H, cfg.KV, cfg.n_batch, cfg.n_shard)
    if key not in _NC_CACHE:
        _NC_CACHE[key] = build(cfg)
    return _NC_CACHE[key]


def run_on_hw(cfg, x, Wq, Wk, Wv, Wo, trace=False):
    nc = get_nc(cfg)
    in_maps = make_inputs(cfg, x, Wq, Wk, Wv, Wo)
    res = run_bass_kernel_spmd(nc, in_maps, list(range(cfg.n_cores)),
                               trace=trace)
    return scatter_out(cfg, [r for r in res.results]), res


def kernel(x, Wq, Wk, Wv, Wo):
    out, _ = run_on_hw(FULL, np.asarray(x), np.asarray(Wq), np.asarray(Wk),
                       np.asarray(Wv), np.asarray(Wo))
    return out



# revision 10
# speedup vs baseline: 1.0676x; 1.0676x over previous
"""GQA causal attention block (x @ Wq/Wk/Wv -> causal GQA attention -> @ Wo)
for Trainium2, SPMD over 8 NeuronCores.

Sharding: 4 batches x 2 query-shards. Core c handles batch c//2 and the
interleaved set of 128-row query tiles {s, s+2, ...} (s = c%2), which
balances the causal-attention triangle between the two shards of a batch.

vs. the v2 kernel (638us):
- x is transposed by the DMA XBAR (dma_start_transpose) straight into
  xqT; the 128 PE transposes + copies of v2 are gone.
- the attention inner loop is software-pipelined: the score matmuls for
  key-pair t+1 are emitted BEFORE the p@V matmuls of pair t, so the PE
  streams through scores/PV back-to-back while the scalar-engine exp of
  pair t runs in the shadow of pair t+1's scores. v2 serialized
  score->exp->PV per pair, idling the PE ~900ns per pair.
- each query tile j's key slots are re-paired so BOTH masked slots (the
  own-diagonal j and the partner-last 8+j) land in one pair, applied
  with a single [128,1024] DVE add of a combined host-built mask.
- q-projection writes 4-query-tile blocks ([128,4,512] tiles) so PSUM
  evacuation is 2 strided copies per psum tile instead of 8.
- q-proj of groups 2/3 is metered into the attention stream with
  deadline quotas (g2 before task 72, g3 before task 108) instead of
  36-yields-then-drain.

Key-slot layout keeps the SPMD program shard-independent: slots 0..7
hold the core's own key tiles (local order), slots 8..15 the
partner's. Query tile j attends over slots {0..j} u {8..8+j}; the
host-provided mask2 makes it causal: mask2[:, 0:512] (own diagonal
slot j) is triangular for both shards, mask2[:, 512:1024] (slot 8+j)
is -inf for shard 0 (future keys) and 0 for shard 1 (past keys). The
partner block's position in the AllGather output is the only
rank-dependent address, supplied per-core as a uint32 element offset
("poff") and used as a runtime DMA offset register.
"""

import sys

for _p in ("/opt/trn_rl_repo", "/root/.axon_site/_ro/trn_rl_repo"):
    if _p not in sys.path:
        sys.path.append(_p)

import numpy as np
import ml_dtypes

import concourse.bacc as bacc
import concourse.bass as bass
import concourse.tile as tile
import concourse.mybir as mybir
from concourse.bass_utils import run_bass_kernel_spmd

F32 = mybir.dt.float32
BF16 = mybir.dt.bfloat16
FP16 = mybir.dt.float16
U32 = mybir.dt.uint32
AF = mybir.ActivationFunctionType
NEG = -1.0e6  # additive mask for disallowed keys (pre-softmax-scale)


class Cfg:
    def __init__(self, T, E, H, KV, n_batch, n_shard):
        self.T, self.E, self.H, self.KV = T, E, H, KV
        self.D = 128
        self.G4 = H // 4             # 4-head kv groups
        self.NE = E // 128           # contraction chunks for projections
        self.n_batch = n_batch
        self.n_shard = n_shard
        self.n_cores = n_batch * n_shard
        self.RQ = T // n_shard       # query rows per core
        self.NJ = self.RQ // 128     # local 128-row query tiles
        self.NLT = self.RQ // 512    # local 512-row chunks
        self.NT = T // 128           # global 128-row tiles
        self.HKV = KV * self.D       # k/v projection width
        self.scale = 1.0 / float(np.sqrt(self.D))
        # per-chunk exchange block: kT (KV heads) + v (4 local tiles)
        self.CCB = (self.KV + 4) * 128  # rows per cc_in buffer


FULL = Cfg(T=2048, E=2048, H=16, KV=4, n_batch=4, n_shard=2)


def build(cfg):
    c = cfg
    nc = bacc.Bacc("TRN2", target_bir_lowering=False, debug=False,
                   num_devices=c.n_cores)

    xq_d = nc.dram_tensor("xq", [c.RQ, c.E], BF16, kind="ExternalInput").ap()
    wq_d = nc.dram_tensor("Wq", [c.E, c.H * c.D], BF16, kind="ExternalInput").ap()
    wk_d = nc.dram_tensor("Wk", [c.E, c.HKV], BF16, kind="ExternalInput").ap()
    wv_d = nc.dram_tensor("Wv", [c.E, c.HKV], BF16, kind="ExternalInput").ap()
    wo_d = nc.dram_tensor("Wo", [c.H * c.D, c.E], BF16,
                          kind="ExternalInput").ap()
    mask_d = nc.dram_tensor("masks", [128, 1024], F32,
                            kind="ExternalInput").ap()
    idb_d = nc.dram_tensor("identb", [128, 128], BF16, kind="ExternalInput").ap()
    onesh_d = nc.dram_tensor("onesh", [128, 128], FP16, kind="ExternalInput").ap()
    poff_d = nc.dram_tensor("poff", [1, 1], U32, kind="ExternalInput").ap()
    o_d = nc.dram_tensor("o", [c.RQ, c.E], F32, kind="ExternalOutput").ap()

    from contextlib import ExitStack
    with tile.TileContext(nc) as tc:
        with ExitStack() as _st:
            def pool(name, bufs, space="SBUF"):
                return _st.enter_context(
                    tc.tile_pool(name=name, bufs=bufs, space=space))
            constp = pool("const", 1)
            xqtp = pool("xqt", c.NE)
            ktp = pool("kts", c.KV * 4)
            vp = pool("vsb", c.NT)
            qbp = pool("qblk", 2 * c.G4)
            ytp = pool("yt", c.G4 * c.NJ)
            wqp = pool("wq", c.NE)
            wkvp = pool("wkv", 6)
            wop = pool("wo", 8)
            smp = pool("sm", 4)
            accp = pool("accp", 4)
            bsbp = pool("bsb", 2)
            osbp = pool("osb", 4)
            pq = pool("pq", 2, space="PSUM")
            pa = pool("pa", 2, space="PSUM")
            py = pool("py", 2, space="PSUM")
            dramp = pool("dram", 1, space="DRAM")

            # --- constants (identb first: the warmup needs it; the rest go
            # on the scalar queue so they don't delay the first x tiles) ---
            identb = constp.tile([128, 128], BF16, tag="identb")
            nc.sync.dma_start(identb[:], idb_d[:])
            mask2 = constp.tile([128, 1024], F32, tag="mask2", name="mask2")
            nc.scalar.dma_start(mask2[:], mask_d[:])
            onesh = constp.tile([128, 128], FP16, tag="onesh")
            nc.scalar.dma_start(onesh[:], onesh_d[:])
            poffs = constp.tile([1, 1], U32, tag="poffs")
            nc.scalar.dma_start(poffs[:], poff_d[:])

            cc_in = [dramp.tile([c.CCB, 512], BF16, name=f"cc_in{lt}",
                                tag=f"cc_in{lt}") for lt in range(c.NLT)]
            cc_out = [dramp.tile([2 * c.CCB, 512], BF16,
                                 name=f"cc_out{lt}",
                                 tag=f"cc_out{lt}") for lt in range(c.NLT)]

            # warm the PE clock-gate during the initial DMA ramp
            pwu = pa.tile([128, 512], BF16, tag="pa", name="pwu")
            for wu in range(24):
                nc.tensor.transpose(pwu[:, (wu % 4) * 128:(wu % 4 + 1) * 128],
                                    identb[:], identb[:])

            # persistent activations
            xqT = [xqtp.tile([128, c.RQ], BF16, tag="xqT", name=f"xqT{e}")
                   for e in range(c.NE)]
            kts = [[ktp.tile([128, 512], BF16, tag="kts", name=f"kts{h}_{q}")
                    for q in range(4)] for h in range(c.KV)]
            v_sb = [vp.tile([128, c.HKV], BF16, tag="v", name=f"v{i}")
                    for i in range(c.NT)]

            # partner block offset (elements) comes from host data
            poff_r = nc.gpsimd.alloc_register("poff_r")
            nc.gpsimd.reg_load(poff_r, poffs[0:1, 0:1])
            poff_v = nc.gpsimd.snap(poff_r, donate=True, min_val=0,
                                    max_val=c.CCB * 512)

            def cc_src(lt, block):
                off = poff_v + block * 128 * 512
                return bass.AP(cc_out[lt].tensor, off, [[512, 128], [1, 512]])

            # ---------------- Phase A: DMA-transpose x + own-half k/v -------
            def phase_a(lt):
                # XBAR-transpose own 512 rows into xqT[e][:, lt*512:(lt+1)*512]
                for e in range(c.NE):
                    nc.sync.dma_start_transpose(
                        xqT[e][:, lt * 512:(lt + 1) * 512],
                        xq_d[lt * 512:(lt + 1) * 512,
                             e * 128:(e + 1) * 128])

                # kT for own rows -> quad lt
                psk = ([pq.tile([128, 512], F32, tag="pq", name=f"psk{h}")
                        for h in range(2)] +
                       [pa.tile([128, 512], F32, tag="pa", name=f"psk{h + 2}")
                        for h in range(2)])
                for e in range(c.NE):
                    wk_t = wkvp.tile([128, c.HKV], BF16, tag="wkv", name="wk_t")
                    nc.gpsimd.dma_start(wk_t[:], wk_d[e * 128:(e + 1) * 128, :])
                    for h in range(c.KV):
                        nc.tensor.matmul(psk[h][:],
                                         wk_t[:, h * 128:(h + 1) * 128],
                                         xqT[e][:, lt * 512:(lt + 1) * 512],
                                         start=(e == 0), stop=(e == c.NE - 1))
                for h in range(c.KV):
                    nc.vector.tensor_copy(kts[h][lt][:], psk[h][:])
                    nc.gpsimd.dma_start(
                        cc_in[lt][h * 128:(h + 1) * 128, :], kts[h][lt][:])

                # v for own rows -> slots 4*lt..4*lt+3
                psv = ([pq.tile([128, c.HKV], F32, tag="pq", name=f"psv{i}")
                        for i in range(2)] +
                       [pa.tile([128, c.HKV], F32, tag="pa", name=f"psv{i + 2}")
                        for i in range(2)])
                for e in range(c.NE):
                    wv_t = wkvp.tile([128, c.HKV], BF16, tag="wkv", name="wv_t")
                    nc.gpsimd.dma_start(wv_t[:], wv_d[e * 128:(e + 1) * 128, :])
                    for i in range(4):
                        nc.tensor.matmul(psv[i][:],
                                         xqT[e][:, lt * 512 + i * 128:
                                                lt * 512 + (i + 1) * 128],
                                         wv_t[:],
                                         start=(e == 0), stop=(e == c.NE - 1))
                for i in range(4):
                    sl = lt * 4 + i
                    nc.vector.tensor_copy(v_sb[sl][:], psv[i][:])
                    nc.gpsimd.dma_start(
                        cc_in[lt][(c.KV + i) * 128:(c.KV + i + 1) * 128, :],
                        v_sb[sl][:])

            def launch_ag(lt):
                nc.gpsimd.collective_compute(
                    "AllGather",
                    mybir.AluOpType.bypass,
                    replica_groups=[[2 * p, 2 * p + 1]
                                    for p in range(c.n_cores // 2)],
                    ins=[cc_in[lt].opt()],
                    outs=[cc_out[lt].opt()],
                )

            def unpack(lt):
                for h in range(c.KV):
                    nc.gpsimd.dma_start(kts[h][2 + lt][:], cc_src(lt, h))
                for i in range(4):
                    nc.gpsimd.dma_start(v_sb[8 + lt * 4 + i][:],
                                        cc_src(lt, c.KV + i))

            phase_a(0)
            launch_ag(0)
            phase_a(1)
            launch_ag(1)
            unpack(0)
            unpack(1)

            # ---------------- q-projection for one group --------------------
            # writes qblk[g][blk]: [128, 4, 512] = (D, query tile jj, 4h*128q)
            # generator: yields once per PE matmul so attention can consume
            # it as PE filler between scalar-bound softmax pairs
            qblk = [[None] * 2 for _ in range(c.G4)]

            def q_proj_gen(g):
                wqt = []
                for e in range(c.NE):
                    w = wqp.tile([128, 512], BF16, tag="wq", name=f"wq{e}")
                    nc.sync.dma_start(
                        w[:], wq_d[e * 128:(e + 1) * 128,
                                   g * 512:(g + 1) * 512])
                    wqt.append(w)
                for blk in range(2):
                    qb = qbp.tile([128, 4, 512], BF16, tag="qb",
                                  name=f"qb{g}_{blk}")
                    # assign eagerly: the scores that read qb are ordered
                    # after the fills by tile deps, not by python time
                    qblk[g][blk] = qb
                    for hp in range(2):
                        psq = [pq.tile([128, 512], F32, tag="pq",
                                       name=f"psq{i}") for i in range(2)]
                        for e in range(c.NE):
                            for hi in range(2):
                                hh = hp * 2 + hi
                                nc.tensor.matmul(
                                    psq[hi][:],
                                    wqt[e][:, hh * 128:(hh + 1) * 128],
                                    xqT[e][:, blk * 512:(blk + 1) * 512],
                                    start=(e == 0), stop=(e == c.NE - 1))
                                if e < c.NE - 1 or hi < 1:
                                    yield
                        # the copies ride with the final matmul's yield so
                        # generator suspension can never strand them after
                        # a consumer emitted by the metering loop
                        for hi in range(2):
                            hh = hp * 2 + hi
                            nc.vector.tensor_copy(
                                qb[:, :, hh * 128:(hh + 1) * 128],
                                psq[hi][:])
                        yield

            def drain(gen):
                if gen is not None:
                    for _ in gen:
                        pass

            # ---------------- attention: flat pipelined task stream ---------
            # task (g, j, p): p == 0 is the "special" pair (own-diagonal slot
            # j, partner-last slot 8+j) masked by one [128,1024] DVE add of
            # mask2; pairs p >= 1 take consecutive slots from
            # [0..j-1] + [8..8+j-1] and need no mask. Scores for task t+1 are
            # emitted before pair t's p@V so the PE never waits for the exp.
            def task_slots(j, p):
                if p == 0:
                    return j, 8 + j
                u = 2 * (p - 1)
                sl0 = u if u < j else 8 + u - j
                sl1 = (u + 1) if (u + 1) < j else 8 + (u + 1) - j
                return sl0, sl1

            tasks = [(g, j, p)
                     for g in range(c.G4)
                     for j in range(c.NJ)
                     for p in range(j + 1)]

            def kslice(g, sl):
                return kts[g][sl // 4][:, (sl % 4) * 128:(sl % 4 + 1) * 128]

            def emit_scores(t):
                g, j, p = t
                sl0, sl1 = task_slots(j, p)
                sct2 = pa.tile([128, 1024], F32, tag="pa", name="sct2")
                qT = qblk[g][j // 4][:, j % 4, :]
                nc.tensor.matmul(sct2[:, 0:512], kslice(g, sl0), qT,
                                 start=True, stop=True)
                nc.tensor.matmul(sct2[:, 512:1024], kslice(g, sl1), qT,
                                 start=True, stop=True)
                return sct2

            psys = {}
            accs = {}

            def emit_post(t, sct2):
                g, j, p = t
                sl0, sl1 = task_slots(j, p)
                if p == 0:
                    nc.vector.tensor_add(sct2[:], sct2[:], mask2[:])
                pbt2 = smp.tile([128, 1024], BF16, tag="pbt", name="pbt2")
                nc.scalar.activation(pbt2[:], sct2[:], AF.Exp, scale=c.scale)
                if p == 0:
                    acc = accp.tile([128, 512], FP16, tag="acc", name="acc")
                    accs[(g, j)] = acc
                    nc.vector.tensor_copy(acc[:], pbt2[:, 0:512])
                else:
                    acc = accs[(g, j)]
                    nc.vector.tensor_add(acc[:], acc[:], pbt2[:, 0:512])
                nc.vector.tensor_add(acc[:], acc[:], pbt2[:, 512:1024])
                if p == 0:
                    psy = py.tile([128, 512], F32, tag="py", name="psy")
                    psys[(g, j)] = psy
                else:
                    psy = psys[(g, j)]
                nc.tensor.matmul(psy[:],
                                 v_sb[sl0][:, g * 128:(g + 1) * 128],
                                 pbt2[:, 0:512],
                                 start=(p == 0), stop=False)
                nc.tensor.matmul(psy[:],
                                 v_sb[sl1][:, g * 128:(g + 1) * 128],
                                 pbt2[:, 512:1024],
                                 start=False, stop=(p == j))

            yT = [[None] * c.NJ for _ in range(c.G4)]

            def emit_epilogue(g, j):
                psums = pa.tile([128, 512], F32, tag="pa", name="psums")
                nc.tensor.matmul(psums[:], onesh[:], accs.pop((g, j))[:],
                                 start=True, stop=True)
                bsb = bsbp.tile([128, 512], F32, tag="bsb", name="bsb")
                nc.vector.reciprocal_approx_fast(bsb[:], psums[:])
                yt = ytp.tile([128, 512], BF16, tag="yT", name=f"yT{g}_{j}")
                nc.vector.tensor_mul(yt[:], psys.pop((g, j))[:], bsb[:])
                yT[g][j] = yt

            # groups 0/1 projected up front (covers the AllGather window);
            # groups 2/3 metered into the attention stream SEQUENTIALLY
            # (they rotate through the same wq slots, so their lifetimes
            # must not overlap): g2's 128 matmuls by task 54 (well before
            # its attention at 72), g3's by task 108
            from itertools import chain as _chain
            _SENT = object()
            drain(q_proj_gen(0))
            drain(q_proj_gen(1))
            fgen = _chain(q_proj_gen(2), q_proj_gen(3))
            fdone = 0

            def ftarget(i):
                if i < 54:
                    return (128 * (i + 1) + 53) // 54
                return 128 + min(128, (128 * (i - 53) + 53) // 54)

            pending_epi = None
            sct_next = emit_scores(tasks[0])
            for i, t in enumerate(tasks):
                sct_cur = sct_next
                if i + 1 < len(tasks):
                    sct_next = emit_scores(tasks[i + 1])
                if pending_epi is not None:
                    emit_epilogue(*pending_epi)
                    pending_epi = None
                emit_post(t, sct_cur)
                g, j, p = t
                if p == j:
                    pending_epi = (g, j)
                while fdone < ftarget(i):
                    if next(fgen, _SENT) is _SENT:
                        fdone = 10**9
                        break
                    fdone += 1
            if pending_epi is not None:
                emit_epilogue(*pending_epi)
            drain(fgen)

            # ---------------- Phase C: o-projection, single pass ------------
            for et in range(c.E // 512):
                pso2 = [pa.tile([128, 1024], F32, tag="pa",
                                name=f"pso2_{i}") for i in range(2)]
                pso = ([pq.tile([128, 512], F32, tag="pq", name=f"pso{i}")
                        for i in range(2)] +
                       [pso2[i][:, half * 512:(half + 1) * 512]
                        for i in range(2) for half in range(2)] +
                       [py.tile([128, 512], F32, tag="py", name=f"pso{i + 6}")
                        for i in range(2)])
                for h in range(c.H):
                    g, hh = divmod(h, 4)
                    wo_t = wop.tile([128, 512], BF16, tag="wo", name="wo_t")
                    nc.gpsimd.dma_start(
                        wo_t[:], wo_d[h * 128:(h + 1) * 128,
                                      et * 512:(et + 1) * 512])
                    for tsub in range(c.NJ):
                        nc.tensor.matmul(
                            pso[tsub][:],
                            yT[g][tsub][:, hh * 128:(hh + 1) * 128],
                            wo_t[:],
                            start=(h == 0), stop=(h == c.H - 1))
                for tsub in range(c.NJ):
                    osb = osbp.tile([128, 512], F32, tag="osb", name="osb")
                    nc.scalar.copy(osb[:], pso[tsub][:])
                    nc.sync.dma_start(o_d[tsub * 128:(tsub + 1) * 128,
                                          et * 512:(et + 1) * 512],
                                      osb[:])

    nc.compile()
    return nc


def make_masks(cfg, s):
    """Additive causal mask in scoresT ([key, query]) orientation, tiled
    4x along the free axis for the 4-head packing.

    mask2[:, 0:512] is added on the own-side diagonal slot (slot j):
    triangular keep k <= q for both shards. mask2[:, 512:1024] is added on
    the partner-side final slot (slot 8+j): for shard 0 the partner tile
    holds future keys (drop all), for shard 1 past keys (keep all).
    """
    r = np.arange(128)
    triT = np.where(r[:, None] <= r[None, :], 0.0, NEG).astype(np.float32)
    out = np.zeros((2, 128, 128), np.float32)
    out[0] = triT
    if s == 0:
        out[1] = NEG
    return np.tile(out, (1, 1, 4)).transpose(1, 0, 2).reshape(128, 1024)


def make_inputs(cfg, x, Wq, Wk, Wv, Wo):
    """Per-core input maps from full tensors (activations/weights in bf16)."""
    bf = ml_dtypes.bfloat16
    ident_b = np.eye(128, dtype=bf)
    ones_h = np.ones((128, 128), np.float16)
    Wqb, Wkb, Wvb, Wob = (np.asarray(w).astype(bf) for w in (Wq, Wk, Wv, Wo))
    in_maps = []
    for cc in range(cfg.n_cores):
        b, s = divmod(cc, cfg.n_shard)
        xb = np.asarray(x[b]).astype(bf)
        xq = np.ascontiguousarray(
            xb.reshape(cfg.T // 128, 128, cfg.E)[s::cfg.n_shard]
            .reshape(cfg.RQ, cfg.E))
        poff = np.array([[((cc & 1) ^ 1) * cfg.CCB * 512]], np.uint32)
        in_maps.append({
            "xq": xq, "Wq": Wqb, "Wk": Wkb, "Wv": Wvb, "Wo": Wob,
            "masks": make_masks(cfg, s),
            "identb": ident_b,
            "onesh": ones_h,
            "poff": poff,
        })
    return in_maps


def scatter_out(cfg, results):
    B = cfg.n_batch
    out = np.empty((B, cfg.T, cfg.E), np.float32)
    for cc in range(cfg.n_cores):
        b, s = divmod(cc, cfg.n_shard)
        out[b].reshape(cfg.T // 128, 128, cfg.E)[s::cfg.n_shard] = \
            results[cc]["o"].reshape(cfg.RQ // 128, 128, cfg.E)
    return out


_NC_CACHE = {}


def get_nc(cfg):
    key = (cfg.T, cfg.E, cfg.H, cfg.KV, cfg.n_batch, cfg.n_shard)
    if key not in _NC_CACHE:
        _NC_CACHE[key] = build(cfg)
    return _NC_CACHE[key]


def run_on_hw(cfg, x, Wq, Wk, Wv, Wo, trace=False):
    nc = get_nc(cfg)
    in_maps = make_inputs(cfg, x, Wq, Wk, Wv, Wo)
    res = run_bass_kernel_spmd(nc, in_maps, list(range(cfg.n_cores)),
                               trace=trace)
    return scatter_out(cfg, [r for r in res.results]), res


def kernel(x, Wq, Wk, Wv, Wo):
    out, _ = run_on_hw(FULL, np.asarray(x), np.asarray(Wq), np.asarray(Wk),
                       np.asarray(Wv), np.asarray(Wo))
    return out


# revision 15
# speedup vs baseline: 1.3584x; 1.2724x over previous
"""GQA causal attention block (x @ Wq/Wk/Wv -> causal GQA attention -> @ Wo)
for Trainium2, SPMD over 8 NeuronCores.

Sharding: 4 batches x 2 query-shards. Core c handles batch c//2 and the
interleaved set of 128-row query tiles {s, s+2, ...} (s = c%2), which
balances the causal-attention triangle between the two shards of a batch.

vs. the v2 kernel (638us):
- x is transposed by the DMA XBAR (dma_start_transpose) straight into
  xqT; the 128 PE transposes + copies of v2 are gone.
- the attention inner loop is software-pipelined: the score matmuls for
  key-pair t+1 are emitted BEFORE the p@V matmuls of pair t, so the PE
  streams through scores/PV back-to-back while the scalar-engine exp of
  pair t runs in the shadow of pair t+1's scores. v2 serialized
  score->exp->PV per pair, idling the PE ~900ns per pair.
- each query tile j's key slots are re-paired so BOTH masked slots (the
  own-diagonal j and the partner-last 8+j) land in one pair, applied
  with a single [128,1024] DVE add of a combined host-built mask.
- q-projection writes 4-query-tile blocks ([128,4,512] tiles) so PSUM
  evacuation is 2 strided copies per psum tile instead of 8.
- q-proj of groups 2/3 is metered into the attention stream with
  deadline quotas (g2 before task 72, g3 before task 108) instead of
  36-yields-then-drain.

Key-slot layout keeps the SPMD program shard-independent: slots 0..7
hold the core's own key tiles (local order), slots 8..15 the
partner's. Query tile j attends over slots {0..j} u {8..8+j}; the
host-provided mask2 makes it causal: mask2[:, 0:512] (own diagonal
slot j) is triangular for both shards, mask2[:, 512:1024] (slot 8+j)
is -inf for shard 0 (future keys) and 0 for shard 1 (past keys). The
partner block's position in the AllGather output is the only
rank-dependent address, supplied per-core as a uint32 element offset
("poff") and used as a runtime DMA offset register.
"""

import sys

for _p in ("/opt/trn_rl_repo", "/root/.axon_site/_ro/trn_rl_repo"):
    if _p not in sys.path:
        sys.path.append(_p)

import numpy as np
import ml_dtypes

import concourse.bacc as bacc
import concourse.bass as bass
import concourse.tile as tile
import concourse.mybir as mybir
from concourse.bass_utils import run_bass_kernel_spmd

F32 = mybir.dt.float32
BF16 = mybir.dt.bfloat16
FP16 = mybir.dt.float16
U32 = mybir.dt.uint32
AF = mybir.ActivationFunctionType
NEG = -1.0e6  # additive mask for disallowed keys (pre-softmax-scale)


class Cfg:
    def __init__(self, T, E, H, KV, n_batch, n_shard):
        self.T, self.E, self.H, self.KV = T, E, H, KV
        self.D = 128
        self.G4 = H // 4             # 4-head kv groups
        self.NE = E // 128           # contraction chunks for projections
        self.n_batch = n_batch
        self.n_shard = n_shard
        self.n_cores = n_batch * n_shard
        self.RQ = T // n_shard       # query rows per core
        self.NJ = self.RQ // 128     # local 128-row query tiles
        self.NLT = self.RQ // 512    # local 512-row chunks
        self.NT = T // 128           # global 128-row tiles
        self.HKV = KV * self.D       # k/v projection width
        self.scale = 1.0 / float(np.sqrt(self.D))
        # per-chunk exchange block: kT (KV heads) + v (4 local tiles)
        self.CCB = (self.KV + 4) * 128  # rows per cc_in buffer


FULL = Cfg(T=2048, E=2048, H=16, KV=4, n_batch=4, n_shard=2)


def build(cfg):
    c = cfg
    nc = bacc.Bacc("TRN2", target_bir_lowering=False, debug=False,
                   num_devices=c.n_cores)

    xq_d = nc.dram_tensor("xq", [c.RQ, c.E], BF16, kind="ExternalInput").ap()
    wq_d = nc.dram_tensor("Wq", [c.E, c.H * c.D], BF16, kind="ExternalInput").ap()
    wk_d = nc.dram_tensor("Wk", [c.E, c.HKV], BF16, kind="ExternalInput").ap()
    wv_d = nc.dram_tensor("Wv", [c.E, c.HKV], BF16, kind="ExternalInput").ap()
    wo_d = nc.dram_tensor("Wo", [c.H * c.D, c.E], BF16,
                          kind="ExternalInput").ap()
    mask_d = nc.dram_tensor("masks", [128, 1024], F32,
                            kind="ExternalInput").ap()
    idb_d = nc.dram_tensor("identb", [128, 128], BF16, kind="ExternalInput").ap()
    onesh_d = nc.dram_tensor("onesh", [128, 128], FP16, kind="ExternalInput").ap()
    poff_d = nc.dram_tensor("poff", [1, 1], U32, kind="ExternalInput").ap()
    o_d = nc.dram_tensor("o", [c.RQ, c.E], F32, kind="ExternalOutput").ap()

    from contextlib import ExitStack
    with tile.TileContext(nc) as tc:
        with ExitStack() as _st:
            def pool(name, bufs, space="SBUF"):
                return _st.enter_context(
                    tc.tile_pool(name=name, bufs=bufs, space=space))
            constp = pool("const", 1)
            xqtp = pool("xqt", c.NE)
            ktp = pool("kts", c.KV * 4)
            vp = pool("vsb", c.NT)
            qbp = pool("qblk", 2 * c.G4)
            ytp = pool("yt", c.G4 * c.NJ)
            wqp = pool("wq", c.NE)
            wkvp = pool("wkv", 6)
            wop = pool("wo", 8)
            smp = pool("sm", 4)
            accp = pool("accp", 4)
            bsbp = pool("bsb", 2)
            osbp = pool("osb", 4)
            xnp = pool("xn", 8)
            pq = pool("pq", 2, space="PSUM")
            pa = pool("pa", 2, space="PSUM")
            py = pool("py", 2, space="PSUM")
            dramp = pool("dram", 1, space="DRAM")

            # --- constants (identb first: the warmup needs it; the rest go
            # on the scalar queue so they don't delay the first x tiles) ---
            identb = constp.tile([128, 128], BF16, tag="identb")
            nc.sync.dma_start(identb[:], idb_d[:])
            mask2 = constp.tile([128, 1024], F32, tag="mask2", name="mask2")
            nc.scalar.dma_start(mask2[:], mask_d[:])
            onesh = constp.tile([128, 128], FP16, tag="onesh")
            nc.scalar.dma_start(onesh[:], onesh_d[:])
            poffs = constp.tile([1, 1], U32, tag="poffs")
            nc.scalar.dma_start(poffs[:], poff_d[:])

            cc_in = [dramp.tile([c.CCB, 512], BF16, name=f"cc_in{lt}",
                                tag=f"cc_in{lt}") for lt in range(c.NLT)]
            cc_out = [dramp.tile([2 * c.CCB, 512], BF16,
                                 name=f"cc_out{lt}",
                                 tag=f"cc_out{lt}") for lt in range(c.NLT)]

            # warm the PE clock-gate during the initial DMA ramp
            pwu = pa.tile([128, 512], BF16, tag="pa", name="pwu")
            for wu in range(24):
                nc.tensor.transpose(pwu[:, (wu % 4) * 128:(wu % 4 + 1) * 128],
                                    identb[:], identb[:])

            # persistent activations
            xqT = [xqtp.tile([128, c.RQ], BF16, tag="xqT", name=f"xqT{e}")
                   for e in range(c.NE)]
            kts = [[ktp.tile([128, 512], BF16, tag="kts", name=f"kts{h}_{q}")
                    for q in range(4)] for h in range(c.KV)]
            v_sb = [vp.tile([128, c.HKV], BF16, tag="v", name=f"v{i}")
                    for i in range(c.NT)]

            # partner block offset (elements) comes from host data
            poff_r = nc.gpsimd.alloc_register("poff_r")
            nc.gpsimd.reg_load(poff_r, poffs[0:1, 0:1])
            poff_v = nc.gpsimd.snap(poff_r, donate=True, min_val=0,
                                    max_val=c.CCB * 512)

            def cc_src(lt, block):
                off = poff_v + block * 128 * 512
                return bass.AP(cc_out[lt].tensor, off, [[512, 128], [1, 512]])

            # ---------------- Phase A: transposes + own-half k/v ------------
            # The x transposes ride the PE interleaved with the k-projection
            # chains (psk lives in pq+py so ptr can rotate through pa): the
            # k matmul for chunk e follows its transposes immediately instead
            # of waiting for the whole 512-row block to transpose first.
            def phase_a(lt):
                psk = ([pq.tile([128, 512], F32, tag="pq", name=f"psk{h}")
                        for h in range(2)] +
                       [py.tile([128, 512], F32, tag="py", name=f"psk{h + 2}")
                        for h in range(2)])
                def k_mms(e):
                    for h in range(c.KV):
                        nc.tensor.matmul(
                            psk[h][:],
                            wk_ts[e][:, h * 128:(h + 1) * 128],
                            xqT[e][:, lt * 512:(lt + 1) * 512],
                            start=(e == 0), stop=(e == c.NE - 1))

                wk_ts = {}
                prev_e = None
                for qa in range(c.NE // 4):
                    xns = []
                    for i in range(4):
                        xn = xnp.tile([128, 512], BF16, tag="xn",
                                      name=f"xn{i}")
                        nc.sync.dma_start(
                            xn[:], xq_d[lt * 512 + i * 128:
                                        lt * 512 + (i + 1) * 128,
                                        qa * 512:(qa + 1) * 512])
                        xns.append(xn)
                    for eh in range(4):
                        e = qa * 4 + eh
                        wk_t = wkvp.tile([128, c.HKV], BF16, tag="wkv",
                                         name="wk_t")
                        nc.gpsimd.dma_start(wk_t[:],
                                            wk_d[e * 128:(e + 1) * 128, :])
                        wk_ts[e] = wk_t
                        ptr = pa.tile([128, 512], BF16, tag="pa", name="ptr")
                        for i in range(4):
                            nc.tensor.transpose(
                                ptr[:, i * 128:(i + 1) * 128],
                                xns[i][:, eh * 128:(eh + 1) * 128], identb[:])
                        nc.vector.tensor_copy(
                            xqT[e][:, lt * 512:(lt + 1) * 512], ptr[:])
                        # k matmuls run one chunk behind the transposes so
                        # the PE never waits on the xqT evacuation copy
                        if prev_e is not None:
                            k_mms(prev_e)
                            wk_ts.pop(prev_e)
                        prev_e = e
                k_mms(prev_e)
                for h in range(c.KV):
                    nc.vector.tensor_copy(kts[h][lt][:], psk[h][:])
                    nc.gpsimd.dma_start(
                        cc_in[lt][h * 128:(h + 1) * 128, :], kts[h][lt][:])

                # v for own rows -> slots 4*lt..4*lt+3
                psv = ([pq.tile([128, c.HKV], F32, tag="pq", name=f"psv{i}")
                        for i in range(2)] +
                       [py.tile([128, c.HKV], F32, tag="py", name=f"psv{i + 2}")
                        for i in range(2)])
                for e in range(c.NE):
                    wv_t = wkvp.tile([128, c.HKV], BF16, tag="wkv", name="wv_t")
                    nc.gpsimd.dma_start(wv_t[:], wv_d[e * 128:(e + 1) * 128, :])
                    for i in range(4):
                        nc.tensor.matmul(psv[i][:],
                                         xqT[e][:, lt * 512 + i * 128:
                                                lt * 512 + (i + 1) * 128],
                                         wv_t[:],
                                         start=(e == 0), stop=(e == c.NE - 1))
                for i in range(4):
                    sl = lt * 4 + i
                    nc.vector.tensor_copy(v_sb[sl][:], psv[i][:])
                    nc.gpsimd.dma_start(
                        cc_in[lt][(c.KV + i) * 128:(c.KV + i + 1) * 128, :],
                        v_sb[sl][:])

            def launch_ag(lt):
                nc.gpsimd.collective_compute(
                    "AllGather",
                    mybir.AluOpType.bypass,
                    replica_groups=[[2 * p, 2 * p + 1]
                                    for p in range(c.n_cores // 2)],
                    ins=[cc_in[lt].opt()],
                    outs=[cc_out[lt].opt()],
                )

            def unpack(lt):
                for h in range(c.KV):
                    nc.gpsimd.dma_start(kts[h][2 + lt][:], cc_src(lt, h))
                for i in range(4):
                    nc.gpsimd.dma_start(v_sb[8 + lt * 4 + i][:],
                                        cc_src(lt, c.KV + i))

            phase_a(0)
            launch_ag(0)
            phase_a(1)
            launch_ag(1)
            unpack(0)
            unpack(1)

            # ---------------- q-projection for one group --------------------
            # writes qblk[g][blk]: [128, 4, 512] = (D, query tile jj, 4h*128q)
            # generator: yields once per PE matmul so attention can consume
            # it as PE filler between scalar-bound softmax pairs
            qblk = [[None] * 2 for _ in range(c.G4)]

            def q_proj_gen(g):
                wqt = []
                for e in range(c.NE):
                    w = wqp.tile([128, 512], BF16, tag="wq", name=f"wq{e}")
                    nc.sync.dma_start(
                        w[:], wq_d[e * 128:(e + 1) * 128,
                                   g * 512:(g + 1) * 512])
                    wqt.append(w)
                for blk in range(2):
                    qb = qbp.tile([128, 4, 512], BF16, tag="qb",
                                  name=f"qb{g}_{blk}")
                    # assign eagerly: the scores that read qb are ordered
                    # after the fills by tile deps, not by python time
                    qblk[g][blk] = qb
                    for hp in range(2):
                        psq = [pq.tile([128, 512], F32, tag="pq",
                                       name=f"psq{i}") for i in range(2)]
                        for e in range(c.NE):
                            for hi in range(2):
                                hh = hp * 2 + hi
                                nc.tensor.matmul(
                                    psq[hi][:],
                                    wqt[e][:, hh * 128:(hh + 1) * 128],
                                    xqT[e][:, blk * 512:(blk + 1) * 512],
                                    start=(e == 0), stop=(e == c.NE - 1))
                                if e < c.NE - 1 or hi < 1:
                                    yield
                        # the copies ride with the final matmul's yield so
                        # generator suspension can never strand them after
                        # a consumer emitted by the metering loop
                        for hi in range(2):
                            hh = hp * 2 + hi
                            nc.vector.tensor_copy(
                                qb[:, :, hh * 128:(hh + 1) * 128],
                                psq[hi][:])
                        yield

            def drain(gen):
                if gen is not None:
                    for _ in gen:
                        pass

            # ---------------- attention: flat pipelined task stream ---------
            # task (g, j, p): p == 0 is the "special" pair (own-diagonal slot
            # j, partner-last slot 8+j) masked by one [128,1024] DVE add of
            # mask2; pairs p >= 1 take consecutive slots from
            # [0..j-1] + [8..8+j-1] and need no mask. Scores for task t+1 are
            # emitted before pair t's p@V so the PE never waits for the exp.
            def task_slots(j, p):
                if p == 0:
                    return j, 8 + j
                u = 2 * (p - 1)
                sl0 = u if u < j else 8 + u - j
                sl1 = (u + 1) if (u + 1) < j else 8 + (u + 1) - j
                return sl0, sl1

            tasks = [(g, j, p)
                     for g in range(c.G4)
                     for j in range(c.NJ)
                     for p in range(j + 1)]

            def kslice(g, sl):
                return kts[g][sl // 4][:, (sl % 4) * 128:(sl % 4 + 1) * 128]

            def emit_scores(t):
                g, j, p = t
                sl0, sl1 = task_slots(j, p)
                sct2 = pa.tile([128, 1024], F32, tag="pa", name="sct2")
                qT = qblk[g][j // 4][:, j % 4, :]
                nc.tensor.matmul(sct2[:, 0:512], kslice(g, sl0), qT,
                                 start=True, stop=True)
                nc.tensor.matmul(sct2[:, 512:1024], kslice(g, sl1), qT,
                                 start=True, stop=True)
                return sct2

            psys = {}
            accs = {}

            def emit_post(t, sct2):
                g, j, p = t
                sl0, sl1 = task_slots(j, p)
                if p == 0:
                    nc.vector.tensor_add(sct2[:], sct2[:], mask2[:])
                pbt2 = smp.tile([128, 1024], BF16, tag="pbt", name="pbt2")
                nc.scalar.activation(pbt2[:], sct2[:], AF.Exp, scale=c.scale)
                if p == 0:
                    acc = accp.tile([128, 512], FP16, tag="acc", name="acc")
                    accs[(g, j)] = acc
                    nc.vector.tensor_copy(acc[:], pbt2[:, 0:512])
                else:
                    acc = accs[(g, j)]
                    nc.vector.tensor_add(acc[:], acc[:], pbt2[:, 0:512])
                nc.vector.tensor_add(acc[:], acc[:], pbt2[:, 512:1024])
                if p == 0:
                    psy = py.tile([128, 512], F32, tag="py", name="psy")
                    psys[(g, j)] = psy
                else:
                    psy = psys[(g, j)]
                nc.tensor.matmul(psy[:],
                                 v_sb[sl0][:, g * 128:(g + 1) * 128],
                                 pbt2[:, 0:512],
                                 start=(p == 0), stop=False)
                nc.tensor.matmul(psy[:],
                                 v_sb[sl1][:, g * 128:(g + 1) * 128],
                                 pbt2[:, 512:1024],
                                 start=False, stop=(p == j))

            yT = [[None] * c.NJ for _ in range(c.G4)]

            def emit_epilogue(g, j):
                psums = pa.tile([128, 512], F32, tag="pa", name="psums")
                nc.tensor.matmul(psums[:], onesh[:], accs.pop((g, j))[:],
                                 start=True, stop=True)
                bsb = bsbp.tile([128, 512], F32, tag="bsb", name="bsb")
                nc.vector.reciprocal_approx_fast(bsb[:], psums[:])
                yt = ytp.tile([128, 512], BF16, tag="yT", name=f"yT{g}_{j}")
                nc.vector.tensor_mul(yt[:], psys.pop((g, j))[:], bsb[:])
                yT[g][j] = yt

            # only group 0's first query block is projected up front; the
            # remaining 7 half-group blocks are metered into the attention
            # stream as PE filler with one deadline per block (the iteration
            # BEFORE the lookahead scores that first read it). Generators
            # run strictly sequentially (shared wq slots).
            from itertools import chain as _chain
            _SENT = object()
            gen0 = q_proj_gen(0)
            for _ in range(64):
                next(gen0)
            fgen = _chain(gen0, q_proj_gen(1), q_proj_gen(2), q_proj_gen(3))
            fdone = 0

            # cumulative filler-matmul target: 64 more (g0 blk1) by the
            # iteration before task 10 (g0 j=4 scores lookahead), then 64
            # per block at each group/block boundary
            _DL = [(8, 64), (34, 128), (44, 192), (70, 256),
                   (80, 320), (106, 384), (116, 448)]

            def ftarget(i):
                prev_d, prev_c = -1, 0
                for d, cc in _DL:
                    if i <= d:
                        return prev_c + ((cc - prev_c) * (i - prev_d)
                                         + (d - prev_d) - 1) // (d - prev_d)
                    prev_d, prev_c = d, cc
                return _DL[-1][1]

            pending_epi = None
            sct_next = emit_scores(tasks[0])
            for i, t in enumerate(tasks):
                sct_cur = sct_next
                if i + 1 < len(tasks):
                    sct_next = emit_scores(tasks[i + 1])
                if pending_epi is not None:
                    emit_epilogue(*pending_epi)
                    pending_epi = None
                emit_post(t, sct_cur)
                g, j, p = t
                if p == j:
                    pending_epi = (g, j)
                while fdone < ftarget(i):
                    if next(fgen, _SENT) is _SENT:
                        fdone = 10**9
                        break
                    fdone += 1
            if pending_epi is not None:
                emit_epilogue(*pending_epi)
            drain(fgen)

            # ---------------- Phase C: o-projection, single pass ------------
            for et in range(c.E // 512):
                pso2 = [pa.tile([128, 1024], F32, tag="pa",
                                name=f"pso2_{i}") for i in range(2)]
                pso = ([pq.tile([128, 512], F32, tag="pq", name=f"pso{i}")
                        for i in range(2)] +
                       [pso2[i][:, half * 512:(half + 1) * 512]
                        for i in range(2) for half in range(2)] +
                       [py.tile([128, 512], F32, tag="py", name=f"pso{i + 6}")
                        for i in range(2)])
                for h in range(c.H):
                    g, hh = divmod(h, 4)
                    wo_t = wop.tile([128, 512], BF16, tag="wo", name="wo_t")
                    nc.gpsimd.dma_start(
                        wo_t[:], wo_d[h * 128:(h + 1) * 128,
                                      et * 512:(et + 1) * 512])
                    for tsub in range(c.NJ):
                        nc.tensor.matmul(
                            pso[tsub][:],
                            yT[g][tsub][:, hh * 128:(hh + 1) * 128],
                            wo_t[:],
                            start=(h == 0), stop=(h == c.H - 1))
                for tsub in range(c.NJ):
                    osb = osbp.tile([128, 512], F32, tag="osb", name="osb")
                    nc.scalar.copy(osb[:], pso[tsub][:])
                    nc.sync.dma_start(o_d[tsub * 128:(tsub + 1) * 128,
                                          et * 512:(et + 1) * 512],
                                      osb[:])

    nc.compile()
    return nc


def make_masks(cfg, s):
    """Additive causal mask in scoresT ([key, query]) orientation, tiled
    4x along the free axis for the 4-head packing.

    mask2[:, 0:512] is added on the own-side diagonal slot (slot j):
    triangular keep k <= q for both shards. mask2[:, 512:1024] is added on
    the partner-side final slot (slot 8+j): for shard 0 the partner tile
    holds future keys (drop all), for shard 1 past keys (keep all).
    """
    r = np.arange(128)
    triT = np.where(r[:, None] <= r[None, :], 0.0, NEG).astype(np.float32)
    out = np.zeros((2, 128, 128), np.float32)
    out[0] = triT
    if s == 0:
        out[1] = NEG
    return np.tile(out, (1, 1, 4)).transpose(1, 0, 2).reshape(128, 1024)


def make_inputs(cfg, x, Wq, Wk, Wv, Wo):
    """Per-core input maps from full tensors (activations/weights in bf16)."""
    bf = ml_dtypes.bfloat16
    ident_b = np.eye(128, dtype=bf)
    ones_h = np.ones((128, 128), np.float16)
    Wqb, Wkb, Wvb, Wob = (np.asarray(w).astype(bf) for w in (Wq, Wk, Wv, Wo))
    in_maps = []
    for cc in range(cfg.n_cores):
        b, s = divmod(cc, cfg.n_shard)
        xb = np.asarray(x[b]).astype(bf)
        xq = np.ascontiguousarray(
            xb.reshape(cfg.T // 128, 128, cfg.E)[s::cfg.n_shard]
            .reshape(cfg.RQ, cfg.E))
        poff = np.array([[((cc & 1) ^ 1) * cfg.CCB * 512]], np.uint32)
        in_maps.append({
            "xq": xq, "Wq": Wqb, "Wk": Wkb, "Wv": Wvb, "Wo": Wob,
            "masks": make_masks(cfg, s),
            "identb": ident_b,
            "onesh": ones_h,
            "poff": poff,
        })
    return in_maps


def scatter_out(cfg, results):
    B = cfg.n_batch
    out = np.empty((B, cfg.T, cfg.E), np.float32)
    for cc in range(cfg.n_cores):
        b, s = divmod(cc, cfg.n_shard)
        out[b].reshape(cfg.T // 128, 128, cfg.E)[s::cfg.n_shard] = \
            results[cc]["o"].reshape(cfg.RQ // 128, 128, cfg.E)
    return out


_NC_CACHE = {}


def get_nc(cfg):
    key = (cfg.T, cfg.E, cfg.H, cfg.KV, cfg.n_batch, cfg.n_shard)
    if key not in _NC_CACHE:
        _NC_CACHE[key] = build(cfg)
    return _NC_CACHE[key]


def run_on_hw(cfg, x, Wq, Wk, Wv, Wo, trace=False):
    nc = get_nc(cfg)
    in_maps = make_inputs(cfg, x, Wq, Wk, Wv, Wo)
    res = run_bass_kernel_spmd(nc, in_maps, list(range(cfg.n_cores)),
                               trace=trace)
    return scatter_out(cfg, [r for r in res.results]), res


def kernel(x, Wq, Wk, Wv, Wo):
    out, _ = run_on_hw(FULL, np.asarray(x), np.asarray(Wq), np.asarray(Wk),
                       np.asarray(Wv), np.asarray(Wo))
    return out


# revision 18
# speedup vs baseline: 1.4142x; 1.0411x over previous
"""GQA causal attention block (x @ Wq/Wk/Wv -> causal GQA attention -> @ Wo)
for Trainium2, SPMD over 8 NeuronCores.

Sharding: 4 batches x 2 query-shards. Core c handles batch c//2 and the
interleaved set of 128-row query tiles {s, s+2, ...} (s = c%2), which
balances the causal-attention triangle between the two shards of a batch.

vs. the v2 kernel (638us):
- x is transposed by the DMA XBAR (dma_start_transpose) straight into
  xqT; the 128 PE transposes + copies of v2 are gone.
- the attention inner loop is software-pipelined: the score matmuls for
  key-pair t+1 are emitted BEFORE the p@V matmuls of pair t, so the PE
  streams through scores/PV back-to-back while the scalar-engine exp of
  pair t runs in the shadow of pair t+1's scores. v2 serialized
  score->exp->PV per pair, idling the PE ~900ns per pair.
- each query tile j's key slots are re-paired so BOTH masked slots (the
  own-diagonal j and the partner-last 8+j) land in one pair, applied
  with a single [128,1024] DVE add of a combined host-built mask.
- q-projection writes 4-query-tile blocks ([128,4,512] tiles) so PSUM
  evacuation is 2 strided copies per psum tile instead of 8.
- q-proj of groups 2/3 is metered into the attention stream with
  deadline quotas (g2 before task 72, g3 before task 108) instead of
  36-yields-then-drain.

Key-slot layout keeps the SPMD program shard-independent: slots 0..7
hold the core's own key tiles (local order), slots 8..15 the
partner's. Query tile j attends over slots {0..j} u {8..8+j}; the
host-provided mask2 makes it causal: mask2[:, 0:512] (own diagonal
slot j) is triangular for both shards, mask2[:, 512:1024] (slot 8+j)
is -inf for shard 0 (future keys) and 0 for shard 1 (past keys). The
partner block's position in the AllGather output is the only
rank-dependent address, supplied per-core as a uint32 element offset
("poff") and used as a runtime DMA offset register.
"""

import sys

for _p in ("/opt/trn_rl_repo", "/root/.axon_site/_ro/trn_rl_repo"):
    if _p not in sys.path:
        sys.path.append(_p)

import numpy as np
import ml_dtypes

import concourse.bacc as bacc
import concourse.bass as bass
import concourse.tile as tile
import concourse.mybir as mybir
from concourse.bass_utils import run_bass_kernel_spmd

F32 = mybir.dt.float32
BF16 = mybir.dt.bfloat16
FP16 = mybir.dt.float16
U32 = mybir.dt.uint32
AF = mybir.ActivationFunctionType
NEG = -1.0e6  # additive mask for disallowed keys (pre-softmax-scale)


class Cfg:
    def __init__(self, T, E, H, KV, n_batch, n_shard):
        self.T, self.E, self.H, self.KV = T, E, H, KV
        self.D = 128
        self.G4 = H // 4             # 4-head kv groups
        self.NE = E // 128           # contraction chunks for projections
        self.n_batch = n_batch
        self.n_shard = n_shard
        self.n_cores = n_batch * n_shard
        self.RQ = T // n_shard       # query rows per core
        self.NJ = self.RQ // 128     # local 128-row query tiles
        self.NLT = self.RQ // 512    # local 512-row chunks
        self.NT = T // 128           # global 128-row tiles
        self.HKV = KV * self.D       # k/v projection width
        self.scale = 1.0 / float(np.sqrt(self.D))
        # per-chunk exchange block: kT (KV heads) + v (4 local tiles)
        self.CCB = (self.KV + 4) * 128  # rows per cc_in buffer


FULL = Cfg(T=2048, E=2048, H=16, KV=4, n_batch=4, n_shard=2)


def build(cfg):
    c = cfg
    nc = bacc.Bacc("TRN2", target_bir_lowering=False, debug=False,
                   num_devices=c.n_cores)

    xq_d = nc.dram_tensor("xq", [c.RQ, c.E], BF16, kind="ExternalInput").ap()
    wq_d = nc.dram_tensor("Wq", [c.E, c.H * c.D], BF16, kind="ExternalInput").ap()
    wk_d = nc.dram_tensor("Wk", [c.E, c.HKV], BF16, kind="ExternalInput").ap()
    wv_d = nc.dram_tensor("Wv", [c.E, c.HKV], BF16, kind="ExternalInput").ap()
    wo_d = nc.dram_tensor("Wo", [c.H * c.D, c.E], BF16,
                          kind="ExternalInput").ap()
    mask_d = nc.dram_tensor("masks", [128, 1024], F32,
                            kind="ExternalInput").ap()
    idb_d = nc.dram_tensor("identb", [128, 128], BF16, kind="ExternalInput").ap()
    onesh_d = nc.dram_tensor("onesh", [128, 128], FP16, kind="ExternalInput").ap()
    poff_d = nc.dram_tensor("poff", [1, 1], U32, kind="ExternalInput").ap()
    o_d = nc.dram_tensor("o", [c.RQ, c.E], F32, kind="ExternalOutput").ap()

    from contextlib import ExitStack
    with tile.TileContext(nc) as tc:
        with ExitStack() as _st:
            def pool(name, bufs, space="SBUF"):
                return _st.enter_context(
                    tc.tile_pool(name=name, bufs=bufs, space=space))
            constp = pool("const", 1)
            xqtp = pool("xqt", c.NE)
            ktp = pool("kts", c.KV * 4)
            vp = pool("vsb", c.NT)
            qbp = pool("qblk", 2 * c.G4)
            ytp = pool("yt", c.G4 * c.NJ)
            wqp = pool("wq", c.NE)
            wkvp = pool("wkv", 10)
            wop = pool("wo", 16)
            smp = pool("sm", 4)
            accp = pool("accp", 4)
            bsbp = pool("bsb", 2)
            osbp = pool("osb", 4)
            xnp = pool("xn", 8)
            pq = pool("pq", 2, space="PSUM")
            pa = pool("pa", 2, space="PSUM")
            py = pool("py", 2, space="PSUM")
            dramp = pool("dram", 1, space="DRAM")

            # --- constants (identb first: the warmup needs it; the rest go
            # on the scalar queue so they don't delay the first x tiles) ---
            identb = constp.tile([128, 128], BF16, tag="identb")
            nc.sync.dma_start(identb[:], idb_d[:])
            mask2 = constp.tile([128, 1024], F32, tag="mask2", name="mask2")
            nc.scalar.dma_start(mask2[:], mask_d[:])
            onesh = constp.tile([128, 128], FP16, tag="onesh")
            nc.scalar.dma_start(onesh[:], onesh_d[:])
            poffs = constp.tile([1, 1], U32, tag="poffs")
            nc.scalar.dma_start(poffs[:], poff_d[:])

            cc_in = [dramp.tile([c.CCB, 512], BF16, name=f"cc_in{lt}",
                                tag=f"cc_in{lt}") for lt in range(c.NLT)]
            cc_out = [dramp.tile([2 * c.CCB, 512], BF16,
                                 name=f"cc_out{lt}",
                                 tag=f"cc_out{lt}") for lt in range(c.NLT)]

            # warm the PE clock-gate during the initial DMA ramp
            pwu = pa.tile([128, 512], BF16, tag="pa", name="pwu")
            for wu in range(24):
                nc.tensor.transpose(pwu[:, (wu % 4) * 128:(wu % 4 + 1) * 128],
                                    identb[:], identb[:])

            # persistent activations
            xqT = [xqtp.tile([128, c.RQ], BF16, tag="xqT", name=f"xqT{e}")
                   for e in range(c.NE)]
            kts = [[ktp.tile([128, 512], BF16, tag="kts", name=f"kts{h}_{q}")
                    for q in range(4)] for h in range(c.KV)]
            v_sb = [vp.tile([128, c.HKV], BF16, tag="v", name=f"v{i}")
                    for i in range(c.NT)]

            # partner block offset (elements) comes from host data
            poff_r = nc.gpsimd.alloc_register("poff_r")
            nc.gpsimd.reg_load(poff_r, poffs[0:1, 0:1])
            poff_v = nc.gpsimd.snap(poff_r, donate=True, min_val=0,
                                    max_val=c.CCB * 512)

            def cc_src(lt, block):
                off = poff_v + block * 128 * 512
                return bass.AP(cc_out[lt].tensor, off, [[512, 128], [1, 512]])

            # ---------------- Phase A: transposes + own-half k/v ------------
            # The x transposes ride the PE interleaved with the k-projection
            # chains (psk lives in pq+py so ptr can rotate through pa): the
            # k matmul for chunk e follows its transposes immediately instead
            # of waiting for the whole 512-row block to transpose first.
            def phase_a(lt):
                psk = ([pq.tile([128, 512], F32, tag="pq", name=f"psk{h}")
                        for h in range(2)] +
                       [py.tile([128, 512], F32, tag="py", name=f"psk{h + 2}")
                        for h in range(2)])
                def k_mms(e):
                    for h in range(c.KV):
                        nc.tensor.matmul(
                            psk[h][:],
                            wk_ts[e][:, h * 128:(h + 1) * 128],
                            xqT[e][:, lt * 512:(lt + 1) * 512],
                            start=(e == 0), stop=(e == c.NE - 1))

                wk_ts = {}
                prev_e = None
                for qa in range(c.NE // 4):
                    xns = []
                    for i in range(4):
                        xn = xnp.tile([128, 512], BF16, tag="xn",
                                      name=f"xn{i}")
                        nc.sync.dma_start(
                            xn[:], xq_d[lt * 512 + i * 128:
                                        lt * 512 + (i + 1) * 128,
                                        qa * 512:(qa + 1) * 512])
                        xns.append(xn)
                    for eh in range(4):
                        e = qa * 4 + eh
                        wk_t = wkvp.tile([128, c.HKV], BF16, tag="wkv",
                                         name="wk_t")
                        nc.gpsimd.dma_start(wk_t[:],
                                            wk_d[e * 128:(e + 1) * 128, :])
                        wk_ts[e] = wk_t
                        ptr = pa.tile([128, 512], BF16, tag="pa", name="ptr")
                        for i in range(4):
                            nc.tensor.transpose(
                                ptr[:, i * 128:(i + 1) * 128],
                                xns[i][:, eh * 128:(eh + 1) * 128], identb[:])
                        nc.vector.tensor_copy(
                            xqT[e][:, lt * 512:(lt + 1) * 512], ptr[:])
                        # k matmuls run one chunk behind the transposes so
                        # the PE never waits on the xqT evacuation copy
                        if prev_e is not None:
                            k_mms(prev_e)
                            wk_ts.pop(prev_e)
                        prev_e = e
                k_mms(prev_e)
                # issue all wv loads BEFORE the kts evacuations/stores so
                # the v matmuls never wait behind the store queue
                wv_ts = {}
                for e in range(c.NE):
                    wv_t = wkvp.tile([128, c.HKV], BF16, tag="wkv", name="wv_t")
                    nc.gpsimd.dma_start(wv_t[:], wv_d[e * 128:(e + 1) * 128, :])
                    wv_ts[e] = wv_t
                for h in range(c.KV):
                    nc.vector.tensor_copy(kts[h][lt][:], psk[h][:])
                    nc.gpsimd.dma_start(
                        cc_in[lt][h * 128:(h + 1) * 128, :], kts[h][lt][:])

                # v for own rows -> slots 4*lt..4*lt+3
                psv = ([pq.tile([128, c.HKV], F32, tag="pq", name=f"psv{i}")
                        for i in range(2)] +
                       [py.tile([128, c.HKV], F32, tag="py", name=f"psv{i + 2}")
                        for i in range(2)])
                for e in range(c.NE):
                    wv_t = wv_ts.pop(e)
                    for i in range(4):
                        nc.tensor.matmul(psv[i][:],
                                         xqT[e][:, lt * 512 + i * 128:
                                                lt * 512 + (i + 1) * 128],
                                         wv_t[:],
                                         start=(e == 0), stop=(e == c.NE - 1))
                for i in range(4):
                    sl = lt * 4 + i
                    nc.vector.tensor_copy(v_sb[sl][:], psv[i][:])
                    nc.gpsimd.dma_start(
                        cc_in[lt][(c.KV + i) * 128:(c.KV + i + 1) * 128, :],
                        v_sb[sl][:])

            def launch_ag(lt):
                nc.gpsimd.collective_compute(
                    "AllGather",
                    mybir.AluOpType.bypass,
                    replica_groups=[[2 * p, 2 * p + 1]
                                    for p in range(c.n_cores // 2)],
                    ins=[cc_in[lt].opt()],
                    outs=[cc_out[lt].opt()],
                )

            def unpack(lt):
                for h in range(c.KV):
                    nc.gpsimd.dma_start(kts[h][2 + lt][:], cc_src(lt, h))
                for i in range(4):
                    nc.gpsimd.dma_start(v_sb[8 + lt * 4 + i][:],
                                        cc_src(lt, c.KV + i))

            phase_a(0)
            launch_ag(0)
            phase_a(1)
            launch_ag(1)
            unpack(0)
            unpack(1)

            # ---------------- q-projection for one group --------------------
            # writes qblk[g][blk]: [128, 4, 512] = (D, query tile jj, 4h*128q)
            # generator: yields once per PE matmul so attention can consume
            # it as PE filler between scalar-bound softmax pairs
            qblk = [[None] * 2 for _ in range(c.G4)]

            def q_proj_gen(g):
                wqt = []
                for e in range(c.NE):
                    w = wqp.tile([128, 512], BF16, tag="wq", name=f"wq{e}")
                    nc.sync.dma_start(
                        w[:], wq_d[e * 128:(e + 1) * 128,
                                   g * 512:(g + 1) * 512])
                    wqt.append(w)
                for blk in range(2):
                    qb = qbp.tile([128, 4, 512], BF16, tag="qb",
                                  name=f"qb{g}_{blk}")
                    # assign eagerly: the scores that read qb are ordered
                    # after the fills by tile deps, not by python time
                    qblk[g][blk] = qb
                    for hp in range(2):
                        psq = [pq.tile([128, 512], F32, tag="pq",
                                       name=f"psq{i}") for i in range(2)]
                        for e in range(c.NE):
                            for hi in range(2):
                                hh = hp * 2 + hi
                                nc.tensor.matmul(
                                    psq[hi][:],
                                    wqt[e][:, hh * 128:(hh + 1) * 128],
                                    xqT[e][:, blk * 512:(blk + 1) * 512],
                                    start=(e == 0), stop=(e == c.NE - 1))
                                if e < c.NE - 1 or hi < 1:
                                    yield
                        # the copies ride with the final matmul's yield so
                        # generator suspension can never strand them after
                        # a consumer emitted by the metering loop
                        for hi in range(2):
                            hh = hp * 2 + hi
                            nc.vector.tensor_copy(
                                qb[:, :, hh * 128:(hh + 1) * 128],
                                psq[hi][:])
                        yield

            def drain(gen):
                if gen is not None:
                    for _ in gen:
                        pass

            # ---------------- attention: flat pipelined task stream ---------
            # task (g, j, p): p == 0 is the "special" pair (own-diagonal slot
            # j, partner-last slot 8+j) masked by one [128,1024] DVE add of
            # mask2; pairs p >= 1 take consecutive slots from
            # [0..j-1] + [8..8+j-1] and need no mask. Scores for task t+1 are
            # emitted before pair t's p@V so the PE never waits for the exp.
            def task_slots(j, p):
                if p == 0:
                    return j, 8 + j
                u = 2 * (p - 1)
                sl0 = u if u < j else 8 + u - j
                sl1 = (u + 1) if (u + 1) < j else 8 + (u + 1) - j
                return sl0, sl1

            tasks = [(g, j, p)
                     for g in range(c.G4)
                     for j in range(c.NJ)
                     for p in range(j + 1)]

            def kslice(g, sl):
                return kts[g][sl // 4][:, (sl % 4) * 128:(sl % 4 + 1) * 128]

            def emit_scores(t):
                g, j, p = t
                sl0, sl1 = task_slots(j, p)
                sct2 = pa.tile([128, 1024], F32, tag="pa", name="sct2")
                qT = qblk[g][j // 4][:, j % 4, :]
                nc.tensor.matmul(sct2[:, 0:512], kslice(g, sl0), qT,
                                 start=True, stop=True)
                nc.tensor.matmul(sct2[:, 512:1024], kslice(g, sl1), qT,
                                 start=True, stop=True)
                return sct2

            psys = {}
            accs = {}

            def emit_post(t, sct2):
                g, j, p = t
                sl0, sl1 = task_slots(j, p)
                if p == 0:
                    nc.vector.tensor_add(sct2[:], sct2[:], mask2[:])
                pbt2 = smp.tile([128, 1024], BF16, tag="pbt", name="pbt2")
                nc.scalar.activation(pbt2[:], sct2[:], AF.Exp, scale=c.scale)
                if p == 0:
                    acc = accp.tile([128, 512], FP16, tag="acc", name="acc")
                    accs[(g, j)] = acc
                    nc.vector.tensor_copy(acc[:], pbt2[:, 0:512])
                else:
                    acc = accs[(g, j)]
                    nc.vector.tensor_add(acc[:], acc[:], pbt2[:, 0:512])
                nc.vector.tensor_add(acc[:], acc[:], pbt2[:, 512:1024])
                if p == 0:
                    psy = py.tile([128, 512], F32, tag="py", name="psy")
                    psys[(g, j)] = psy
                else:
                    psy = psys[(g, j)]
                nc.tensor.matmul(psy[:],
                                 v_sb[sl0][:, g * 128:(g + 1) * 128],
                                 pbt2[:, 0:512],
                                 start=(p == 0), stop=False)
                nc.tensor.matmul(psy[:],
                                 v_sb[sl1][:, g * 128:(g + 1) * 128],
                                 pbt2[:, 512:1024],
                                 start=False, stop=(p == j))

            yT = [[None] * c.NJ for _ in range(c.G4)]

            def emit_epilogue(g, j):
                psums = pa.tile([128, 512], F32, tag="pa", name="psums")
                nc.tensor.matmul(psums[:], onesh[:], accs.pop((g, j))[:],
                                 start=True, stop=True)
                bsb = bsbp.tile([128, 512], F32, tag="bsb", name="bsb")
                nc.vector.reciprocal_approx_fast(bsb[:], psums[:])
                yt = ytp.tile([128, 512], BF16, tag="yT", name=f"yT{g}_{j}")
                nc.vector.tensor_mul(yt[:], psys.pop((g, j))[:], bsb[:])
                yT[g][j] = yt

            # only group 0's first query block is projected up front; the
            # remaining 7 half-group blocks are metered into the attention
            # stream as PE filler with one deadline per block (the iteration
            # BEFORE the lookahead scores that first read it). Generators
            # run strictly sequentially (shared wq slots).
            from itertools import chain as _chain
            _SENT = object()
            gen0 = q_proj_gen(0)
            for _ in range(64):
                next(gen0)
            fgen = _chain(gen0, q_proj_gen(1), q_proj_gen(2), q_proj_gen(3))
            fdone = 0

            # cumulative filler-matmul target: 64 more (g0 blk1) by the
            # iteration before task 10 (g0 j=4 scores lookahead), then 64
            # per block at each group/block boundary
            _DL = [(8, 64), (34, 128), (44, 192), (70, 256),
                   (80, 320), (106, 384), (116, 448)]

            def ftarget(i):
                prev_d, prev_c = -1, 0
                for d, cc in _DL:
                    if i <= d:
                        return prev_c + ((cc - prev_c) * (i - prev_d)
                                         + (d - prev_d) - 1) // (d - prev_d)
                    prev_d, prev_c = d, cc
                return _DL[-1][1]

            pending_epi = None
            sct_next = emit_scores(tasks[0])
            for i, t in enumerate(tasks):
                sct_cur = sct_next
                if i + 1 < len(tasks):
                    sct_next = emit_scores(tasks[i + 1])
                if pending_epi is not None:
                    emit_epilogue(*pending_epi)
                    pending_epi = None
                emit_post(t, sct_cur)
                g, j, p = t
                if p == j:
                    pending_epi = (g, j)
                while fdone < ftarget(i):
                    if next(fgen, _SENT) is _SENT:
                        fdone = 10**9
                        break
                    fdone += 1
            if pending_epi is not None:
                emit_epilogue(*pending_epi)
            drain(fgen)

            # ---------------- Phase C: o-projection ------------------------
            # 8 half-passes of 4 query tiles, alternating two disjoint
            # 4-bank PSUM sets so each pass's evacuation overlaps the next
            # pass's matmuls. Each et's Wo tiles are loaded once and shared
            # by its two half-passes (wop holds all 16).
            for et in range(c.E // 512):
                wo_ts = []
                for h in range(c.H):
                    wo_t = wop.tile([128, 512], BF16, tag="wo", name="wo_t")
                    nc.gpsimd.dma_start(
                        wo_t[:], wo_d[h * 128:(h + 1) * 128,
                                      et * 512:(et + 1) * 512])
                    wo_ts.append(wo_t)
                for half in range(2):
                    if half == 0:
                        pso2 = pa.tile([128, 1024], F32, tag="pa",
                                       name="pso2")
                        pso = ([pq.tile([128, 512], F32, tag="pq",
                                        name=f"pso{i}") for i in range(2)] +
                               [pso2[:, hf * 512:(hf + 1) * 512]
                                for hf in range(2)])
                    else:
                        pso2 = pa.tile([128, 1024], F32, tag="pa",
                                       name="pso2")
                        pso = ([pso2[:, hf * 512:(hf + 1) * 512]
                                for hf in range(2)] +
                               [py.tile([128, 512], F32, tag="py",
                                        name=f"pso{i + 6}")
                                for i in range(2)])
                    for h in range(c.H):
                        g, hh = divmod(h, 4)
                        for ts4 in range(4):
                            tsub = half * 4 + ts4
                            nc.tensor.matmul(
                                pso[ts4][:],
                                yT[g][tsub][:, hh * 128:(hh + 1) * 128],
                                wo_ts[h][:],
                                start=(h == 0), stop=(h == c.H - 1))
                    for ts4 in range(4):
                        tsub = half * 4 + ts4
                        osb = osbp.tile([128, 512], F32, tag="osb",
                                        name="osb")
                        nc.scalar.copy(osb[:], pso[ts4][:])
                        nc.sync.dma_start(o_d[tsub * 128:(tsub + 1) * 128,
                                              et * 512:(et + 1) * 512],
                                          osb[:])

    nc.compile()
    return nc


def make_masks(cfg, s):
    """Additive causal mask in scoresT ([key, query]) orientation, tiled
    4x along the free axis for the 4-head packing.

    mask2[:, 0:512] is added on the own-side diagonal slot (slot j):
    triangular keep k <= q for both shards. mask2[:, 512:1024] is added on
    the partner-side final slot (slot 8+j): for shard 0 the partner tile
    holds future keys (drop all), for shard 1 past keys (keep all).
    """
    r = np.arange(128)
    triT = np.where(r[:, None] <= r[None, :], 0.0, NEG).astype(np.float32)
    out = np.zeros((2, 128, 128), np.float32)
    out[0] = triT
    if s == 0:
        out[1] = NEG
    return np.tile(out, (1, 1, 4)).transpose(1, 0, 2).reshape(128, 1024)


def make_inputs(cfg, x, Wq, Wk, Wv, Wo):
    """Per-core input maps from full tensors (activations/weights in bf16)."""
    bf = ml_dtypes.bfloat16
    ident_b = np.eye(128, dtype=bf)
    ones_h = np.ones((128, 128), np.float16)
    Wqb, Wkb, Wvb, Wob = (np.asarray(w).astype(bf) for w in (Wq, Wk, Wv, Wo))
    in_maps = []
    for cc in range(cfg.n_cores):
        b, s = divmod(cc, cfg.n_shard)
        xb = np.asarray(x[b]).astype(bf)
        xq = np.ascontiguousarray(
            xb.reshape(cfg.T // 128, 128, cfg.E)[s::cfg.n_shard]
            .reshape(cfg.RQ, cfg.E))
        poff = np.array([[((cc & 1) ^ 1) * cfg.CCB * 512]], np.uint32)
        in_maps.append({
            "xq": xq, "Wq": Wqb, "Wk": Wkb, "Wv": Wvb, "Wo": Wob,
            "masks": make_masks(cfg, s),
            "identb": ident_b,
            "onesh": ones_h,
            "poff": poff,
        })
    return in_maps


def scatter_out(cfg, results):
    B = cfg.n_batch
    out = np.empty((B, cfg.T, cfg.E), np.float32)
    for cc in range(cfg.n_cores):
        b, s = divmod(cc, cfg.n_shard)
        out[b].reshape(cfg.T // 128, 128, cfg.E)[s::cfg.n_shard] = \
            results[cc]["o"].reshape(cfg.RQ // 128, 128, cfg.E)
    return out


_NC_CACHE = {}


def get_nc(cfg):
    key = (cfg.T, cfg.E, cfg.H, cfg.KV, cfg.n_batch, cfg.n_shard)
    if key not in _NC_CACHE:
        _NC_CACHE[key] = build(cfg)
    return _NC_CACHE[key]


def run_on_hw(cfg, x, Wq, Wk, Wv, Wo, trace=False):
    nc = get_nc(cfg)
    in_maps = make_inputs(cfg, x, Wq, Wk, Wv, Wo)
    res = run_bass_kernel_spmd(nc, in_maps, list(range(cfg.n_cores)),
                               trace=trace)
    return scatter_out(cfg, [r for r in res.results]), res


def kernel(x, Wq, Wk, Wv, Wo):
    out, _ = run_on_hw(FULL, np.asarray(x), np.asarray(Wq), np.asarray(Wk),
                       np.asarray(Wv), np.asarray(Wo))
    return out
